# revision 101
# baseline (speedup 1.0000x reference)
"""Causal multi-head attention on 8 Trainium2 NeuronCores — v3.

Tensor-parallel over heads (16 heads / 8 cores = 2 heads per core).
Each core: full activations x^T (bf16, [d_model, B*S]), column-shard of
Wq/Wk/Wv ([1024, 128], wq pre-scaled by 1/8), row-shard of Wo ([128, 1024]).
Host sums the 8 partial outputs (contraction-sharded output projection).

v3 changes vs v2:
  - cc ([feat, tok]) built by ONE hwdge DMA-transpose per block instead of
    8 PE transposes + a DVE copy; av/ccp switch to qc-major layout
    [128, 4qc, 2h, 64] so the 512-col transpose lands directly in
    [feat, (qc q)] order.
  - den moves to its own PSUM bank ([128, 4, 2] f32).
  - Q bias applied on DVE (scalar engine stays pure exp).
  - Proj drip rescheduled: batch-1 K/V tiles drip INSIDE block (1,3)
    right before the groups that need them, so the PE stays fed through
    the exp-bound batch-1 blocks.
  - First x/wq/wk slices split across both DMA queues for a faster start.
"""

import os
import sys

for p in ("/opt/trn_rl_repo", "/root/.axon_site/_ro/trn_rl_repo"):
    if os.path.isdir(p) and p not in sys.path:
        sys.path.append(p)

import numpy as np
import ml_dtypes

import concourse.bass as bass
import concourse.bacc as bacc
import concourse.mybir as mybir
import concourse.tile as tile
from concourse.bass_utils import run_bass_kernel_spmd

BF16 = mybir.dt.bfloat16
F32 = mybir.dt.float32
NP_BF16 = ml_dtypes.bfloat16

D_MODEL = 1024
NUM_HEADS = 16
D_K = 64
B = 2
S = 2048
T = B * S            # 4096 tokens
N_CORES = 8
FPC = 128            # features per core (2 heads x 64)
N_QT = S // 512      # 4 q-tiles of 512 per batch
N_KT = S // 128      # 16 k-tiles of 128 per batch
KC = D_MODEL // 128  # 8 contraction chunks for the projections

_AluOp = mybir.AluOpType
_Act = mybir.ActivationFunctionType


def build_nc():
    nc = bacc.Bacc()

    xT = nc.declare_dram_parameter("xT", [D_MODEL, T], BF16, isOutput=False)
    wq = nc.declare_dram_parameter("wq", [128, KC * FPC], BF16, isOutput=False)
    wk = nc.declare_dram_parameter("wk", [128, KC * FPC], BF16, isOutput=False)
    wv = nc.declare_dram_parameter("wv", [128, KC * FPC], BF16, isOutput=False)
    wo = nc.declare_dram_parameter("wo", [FPC, D_MODEL], BF16, isOutput=False)
    bq = nc.declare_dram_parameter("bq", [FPC, 1], F32, isOutput=False)
    bk = nc.declare_dram_parameter("bk", [FPC, 1], F32, isOutput=False)
    bv = nc.declare_dram_parameter("bv", [1, FPC], F32, isOutput=False)
    cmask = nc.declare_dram_parameter("cmask", [128, 128], BF16, isOutput=False)
    ident = nc.declare_dram_parameter("ident", [128, 128], BF16, isOutput=False)
    out = nc.declare_dram_parameter("out", [T, D_MODEL], BF16, isOutput=True)

    with tile.TileContext(nc) as tc:
        with (
            tc.tile_pool(name="persist", bufs=1) as persist,
            tc.tile_pool(name="at_pool", bufs=18) as at_pool,
            tc.tile_pool(name="ccp_pool", bufs=4) as ccp_pool,
            tc.tile_pool(name="ccs_pool", bufs=4) as ccs_pool,
            tc.tile_pool(name="rt_pool", bufs=4) as rt_pool,
            tc.tile_pool(name="ob_pool", bufs=8) as ob_pool,
        ):
            # ---------- persistent SBUF ----------

            # The first projection matmuls need wq/wk chunk 0 and
            # xt[0][0][:, ti*512...]; split those leading slices across
            # both DMA queues so the PE can start ~2.5us in.
            xt = [[persist.tile([128, S], BF16, tag=f"xt{c}_{bb}", name=f"xt{c}_{bb}")
                   for bb in range(B)] for c in range(KC)]

            wq_sb = persist.tile([128, KC, FPC], BF16, tag="wq", name="wq")
            wk_sb = persist.tile([128, KC, FPC], BF16, tag="wk", name="wk")
            wv_sb = persist.tile([128, KC, FPC], BF16, tag="wv", name="wv")

            # gpsimd queue: x chunk 0 leading slices; sync queue: wq/wk
            # chunk 0 — the two queues' fixed costs overlap so the first
            # projection matmul can start ~2.6us in
            wq_fl = wq_sb[:, :, :].rearrange("p c f -> p (c f)")
            wk_fl = wk_sb[:, :, :].rearrange("p c f -> p (c f)")
            wv_fl = wv_sb[:, :, :].rearrange("p c f -> p (c f)")
            nc.gpsimd.dma_start(out=xt[0][0][:, 0:512], in_=xT[0:128, 0:512])
            nc.sync.dma_start(out=wq_fl[:, 0:2 * FPC], in_=wq[:, 0:2 * FPC])
            nc.sync.dma_start(out=wk_fl[:, 0:2 * FPC], in_=wk[:, 0:2 * FPC])
            nc.gpsimd.dma_start(out=xt[0][0][:, 512:1024],
                                in_=xT[0:128, 512:1024])
            nc.gpsimd.dma_start(out=xt[0][0][:, 1024:2048],
                                in_=xT[0:128, 1024:2048])
            nc.sync.dma_start(out=wq_fl[:, 2 * FPC:], in_=wq[:, 2 * FPC:])
            nc.sync.dma_start(out=wk_fl[:, 2 * FPC:], in_=wk[:, 2 * FPC:])
            # split b0 chunk loads across both queues: gpsimd's software
            # descriptor-gen chain (~1us per DMA) otherwise serializes
            # chunk availability
            for c in range(1, KC):
                if c in (1, 2):
                    nc.sync.dma_start(
                        out=xt[c][0], in_=xT[c * 128:(c + 1) * 128, 0:S],
                    )
                else:
                    nc.gpsimd.dma_start(
                        out=xt[c][0], in_=xT[c * 128:(c + 1) * 128, 0:S],
                    )

            nc.sync.dma_start(out=wv_fl[:, 0:4 * FPC], in_=wv[:, 0:4 * FPC])
            nc.sync.dma_start(out=wv_fl[:, 4 * FPC:], in_=wv[:, 4 * FPC:])
            wo_sb = persist.tile([128, D_MODEL], BF16, tag="wo")
            nc.sync.dma_start(out=wo_sb, in_=wo[:, :])

            bq_sb = persist.tile([128, 1], F32, tag="bq")
            nc.sync.dma_start(out=bq_sb, in_=bq[:, :])
            bk_sb = persist.tile([128, 1], F32, tag="bk")
            nc.sync.dma_start(out=bk_sb, in_=bk[:, :])
            bv_sb = persist.tile([128, FPC], F32, tag="bv")
            nc.gpsimd.dma_start(out=bv_sb, in_=bv.ap().to_broadcast([128, FPC]))

            oz_sb = persist.tile([128, 2], BF16, tag="oz")
            nc.vector.memset(oz_sb[:, 0:1], 1.0)
            nc.vector.memset(oz_sb[:, 1:2], 0.0)
            mask_sb = persist.tile([128, 128], BF16, tag="cmask")
            nc.sync.dma_start(out=mask_sb, in_=cmask[:, :])
            ident_sb = persist.tile([128, 128], BF16, tag="ident")
            nc.sync.dma_start(out=ident_sb, in_=ident[:, :])

            qt_tiles = [persist.tile([128, 512], BF16, tag=f"qt{i}", name=f"qt{i}")
                        for i in range(T // 512)]
            kt_tiles = [persist.tile([128, 512], BF16, tag=f"kt{i}", name=f"kt{i}")
                        for i in range(T // 512)]
            v_sb = [persist.tile([128, 130], BF16, tag=f"v{g}", name=f"v{g}")
                    for g in range(T // 128)]

            for c in range(KC):
                nc.gpsimd.dma_start(
                    out=xt[c][1], in_=xT[c * 128:(c + 1) * 128, S:2 * S],
                )

            # ---- startup: batch-0 Q/K projections, chunk-outer so the PE
            # starts as soon as x chunk 0 lands (8 psum banks, then freed)
            # ones columns for the AV denominator, one strided memset per
            # v tile, emitted here so the DVE queue is clear at t=0
            for g in range(T // 128):
                nc.vector.memset(
                    v_sb[g][:, :].rearrange("p (a b) -> p a b", a=2)[:, :, 64:65],
                    1.0)

            with tc.tile_pool(name="startup", bufs=1, space="PSUM") as sup:
                pq0 = [sup.tile([128, 512], F32, tag=f"spq{i}", name=f"spq{i}")
                       for i in range(4)]
                pk0 = [sup.tile([128, 512], F32, tag=f"spk{i}", name=f"spk{i}")
                       for i in range(4)]


                def evac_qk(kind, ti):
                    # q on Act, k on DVE: startup evacs split across engines
                    if kind == "q":
                        nc.scalar.add(qt_tiles[ti], pq0[ti], bq_sb[:, 0:1])
                    else:
                        nc.vector.tensor_scalar(
                            out=kt_tiles[ti], in0=pk0[ti], scalar1=bk_sb[:, 0:1],
                            scalar2=None, op0=_AluOp.add,
                        )

                for c in range(KC - 1):
                    for ti in range(4):
                        sl = slice(ti * 512, ti * 512 + 512)
                        nc.tensor.matmul(
                            pq0[ti], lhsT=wq_sb[:, c, :], rhs=xt[c][0][:, sl],
                            start=(c == 0), stop=False,
                        )
                        nc.tensor.matmul(
                            pk0[ti], lhsT=wk_sb[:, c, :], rhs=xt[c][0][:, sl],
                            start=(c == 0), stop=False,
                        )
                c = KC - 1
                for kind, ti in [("k", 0), ("q", 3), ("q", 0), ("q", 1),
                                 ("q", 2), ("k", 1), ("k", 2), ("k", 3)]:
                    sl = slice(ti * 512, ti * 512 + 512)
                    pt, wt = (pq0, wq_sb) if kind == "q" else (pk0, wk_sb)
                    nc.tensor.matmul(
                        pt[ti], lhsT=wt[:, c, :], rhs=xt[c][0][:, sl],
                        start=False, stop=True,
                    )
                    evac_qk(kind, ti)

            with (
                tc.tile_pool(name="scp", bufs=2, space="PSUM") as scp,   # 4 banks
                tc.tile_pool(name="avp", bufs=1, space="PSUM") as avp,   # 1 bank
                tc.tile_pool(name="ctp", bufs=1, space="PSUM") as ctp,   # 1 bank
                tc.tile_pool(name="mmp", bufs=2, space="PSUM") as mmp,   # 2 banks
            ):
                def proj_qk_half(kind, ti, hf):
                    # half a q/k tile (256 tokens): ~0.85us of PE filler
                    bb, loc = ti // 4, (ti % 4) * 512 + hf * 256
                    sl = slice(loc, loc + 256)
                    w_sb, b_sb, dst = ((wq_sb, bq_sb, qt_tiles) if kind == "q"
                                       else (wk_sb, bk_sb, kt_tiles))
                    pp = mmp.tile([128, 512], F32, tag="po", name="pp")
                    for c in range(KC):
                        nc.tensor.matmul(
                            pp[:, 0:256], lhsT=w_sb[:, c, :],
                            rhs=xt[c][bb][:, sl],
                            start=(c == 0), stop=(c == KC - 1),
                        )
                    nc.vector.tensor_scalar(
                        out=dst[ti][:, hf * 256:hf * 256 + 256],
                        in0=pp[:, 0:256], scalar1=b_sb[:, 0:1],
                        scalar2=None, op0=_AluOp.add,
                    )

                def proj_v2(g2):
                    # 2 consecutive 128-token groups: ~0.85us of PE filler
                    bb = g2 // 8
                    pv = mmp.tile([128, 512], F32, tag="po", name="pv")
                    for c in range(KC):
                        for gi in range(2):
                            g = g2 * 2 + gi
                            loc = (g % N_KT) * 128
                            nc.tensor.matmul(
                                pv[:, gi * 128:(gi + 1) * 128],
                                lhsT=xt[c][bb][:, loc:loc + 128],
                                rhs=wv_sb[:, c, :],
                                start=(c == 0 and gi == 0),
                                stop=(c == KC - 1),
                                skip_group_check=True,
                            )
                    for gi in range(2):
                        g = g2 * 2 + gi
                        nc.vector.tensor_tensor(
                            out=v_sb[g][:, :].rearrange("p (h d) -> p h d", h=2)[:, :, 0:64],
                            in0=pv[:, gi * 128:(gi + 1) * 128].rearrange(
                                "p (h d) -> p h d", h=2),
                            in1=bv_sb[:, :].rearrange("p (h d) -> p h d", h=2),
                            op=_AluOp.add,
                        )

                def score_exp(b, qt, h, grp):
                    """Scores + exp + corner masks for one 2-k-tile group.
                    Returns (at_tile, rels)."""
                    hsl = slice(h * 64, (h + 1) * 64)
                    sc = scp.tile([128, 1024], F32, tag="sc", name="sc")
                    at = at_pool.tile([128, 1024], BF16, tag="at", name="at")
                    rels = []
                    for j in range(2):
                        ki = grp * 2 + j
                        rel = ki * 128 - qt * 512
                        r = rel // 128 if rel >= 0 else -1
                        rels.append(r)
                        kt_i = b * 4 + ki // 4
                        ko = (ki % 4) * 128
                        q0 = r * 128 if r > 0 else 0
                        nc.tensor.matmul(
                            sc[:, j * 512 + q0:(j + 1) * 512],
                            lhsT=kt_tiles[kt_i][hsl, ko:ko + 128],
                            rhs=qt_tiles[b * 4 + qt][hsl, q0:512],
                            start=True, stop=True,
                        )
                    if rels[1] <= 0:
                        nc.scalar.activation(out=at, in_=sc, func=_Act.Exp)
                    elif rels[0] == 0:
                        # one exp across both ranges; cols [512,640) are
                        # garbage but never read downstream
                        nc.scalar.activation(out=at, in_=sc, func=_Act.Exp)
                    else:
                        q0a = rels[0] * 128 if rels[0] > 0 else 0
                        nc.scalar.activation(
                            out=at[:, q0a:512], in_=sc[:, q0a:512],
                            func=_Act.Exp)
                        q0b = 512 + rels[1] * 128
                        nc.scalar.activation(
                            out=at[:, q0b:1024], in_=sc[:, q0b:1024],
                            func=_Act.Exp)
                    for j in range(2):
                        r = rels[j]
                        if r >= 0:
                            c0 = j * 512 + r * 128
                            nc.vector.tensor_tensor(
                                out=at[:, c0:c0 + 128],
                                in0=at[:, c0:c0 + 128],
                                in1=mask_sb,
                                op=_AluOp.mult,
                            )
                    return at, rels

                def av_group(b, h, av, den, at, rels, grp):
                    for j in range(2):
                        ki = grp * 2 + j
                        r = rels[j]
                        g = b * N_KT + ki
                        for qc in range(max(r, 0), 4):
                            sl = slice(j * 512 + qc * 128,
                                       j * 512 + (qc + 1) * 128)
                            nc.tensor.matmul(
                                av[:, qc, h, :],
                                lhsT=at[:, sl],
                                rhs=v_sb[g][:, h * 65:h * 65 + 64],
                                start=(h == 0 and ki == 0 and qc == 0),
                                stop=(r >= 0 and qc == r),
                                skip_group_check=True,
                            )
                            nc.tensor.matmul(
                                den[:, qc, h:h + 1],
                                lhsT=at[:, sl],
                                rhs=oz_sb[:, 0:1],
                                start=False,
                                stop=(r >= 0 and qc == r),
                                skip_group_check=True,
                            )

                def norm_c(av, den):
                    rt = rt_pool.tile([128, 4, 2, 1], F32, tag="rt", name="rt")
                    nc.vector.reciprocal(rt, den[:, :, :, None])
                    ccp = ccp_pool.tile([128, 4, 2, 64], BF16, tag="ccp",
                                        name="ccp")
                    nc.vector.tensor_tensor(
                        out=ccp, in0=av,
                        in1=rt[:, :, :, :].to_broadcast([128, 4, 2, 64]),
                        op=_AluOp.mult,
                    )
                    return ccp

                def attention(b, qt, fillers, act=False, pop_start=0,
                              store_q=None, final=False):
                    """One (batch, 512-q-tile) attention block, both heads,
                    h0/h1 score+exp chains interleaved.  Fillers are
                    (callable, due_iter|None); units due by iteration g are
                    emitted right after iteration g-1's score matmuls so the
                    tiles they write are ready for g's reads.  Returns
                    deferred units: [dma-transpose, oproj x4] to drip into
                    the next block."""
                    tok0 = b * S + qt * 512
                    sqs = store_q or [nc.sync]
                    s_i = [0]
                    nk = (qt + 1) * 4          # visible 128-k-tiles
                    ng = nk // 2
                    av0 = avp.tile([128, 4, 2, 64], F32, tag="av", name="av0")
                    ct528 = ctp.tile([128, 528], BF16, tag="ct", name="ct")
                    ct = ct528[:, 0:512]
                    den = ct528[:, 512:528].bitcast(F32).rearrange(
                        "p (a b) -> p a b", a=4)

                    def run_due(limit):
                        popped = 0.0
                        i = 0
                        while i < len(fillers):
                            f, cost, due = fillers[i]
                            if due is not None and due <= limit:
                                fillers.pop(i)
                                f()
                                popped += cost
                            else:
                                i += 1
                        return popped

                    def est(rels):
                        # (act_ns, pe_ns) for one group's two heads
                        cols = sum(512 - max(r, 0) * 128 for r in rels)
                        ninst = 2 if (rels[0] > 0 and rels[1] > 0) else 1
                        act = 2 * (0.833 * cols + 185 * ninst)
                        pe = 0.8333 * cols          # scores, both heads
                        return act, pe

                    av_pe = 0.0                      # AV cols of prev group
                    deficit = 0.0
                    prev = None
                    for grp in range(ng):
                        run_due(grp)  # safety: anything this grp's reads need
                        at0, rels0 = score_exp(b, qt, 0, grp)
                        at1, rels1 = score_exp(b, qt, 1, grp)
                        a_ns, p_ns = est(rels0)
                        deficit += a_ns - p_ns - av_pe
                        av_pe = 2 * 0.4167 * 65 * sum(
                            4 - max(r, 0) for r in rels0)
                        deficit -= run_due(grp + 1)
                        if grp < pop_start:
                            deficit = min(deficit, 0.0)
                        while deficit > -400 and fillers:
                            f, cost, _due = fillers.pop(0)
                            f()
                            deficit -= cost
                        if grp == min(1, ng - 1):
                            # zero-init den col 0; start=True resets the
                            # bank's accumulation-group tracking.  Emitted as
                            # late as possible so it doesn't stall the PE on
                            # the previous block's ct-bank read.
                            dfl = den[:, :, :].rearrange("p a b -> p (a b)")
                            nc.tensor.matmul(
                                dfl[:, 0:1], lhsT=mask_sb, rhs=oz_sb[:, 1:2],
                                start=True, stop=False, skip_group_check=True,
                            )
                        # AV lags one group so it never waits on its own exp
                        if prev is not None:
                            av_group(b, 0, av0, den, prev[0][0], prev[0][1], grp - 1)
                            av_group(b, 1, av0, den, prev[1][0], prev[1][1], grp - 1)
                        prev = ((at0, rels0), (at1, rels1))
                    av_group(b, 0, av0, den, prev[0][0], prev[0][1], ng - 1)
                    av_group(b, 1, av0, den, prev[1][0], prev[1][1], ng - 1)
                    for f, _cost, _due in fillers:
                        f()
                    if final:
                        # split norm so the first transpose pair starts as
                        # soon as the first half of ccp is normalized
                        rt = rt_pool.tile([128, 4, 2, 1], F32, tag="rt",
                                          name="rt")
                        nc.vector.reciprocal(rt, den[:, :, :, None])
                        ccp0 = ccp_pool.tile([128, 4, 2, 64], BF16,
                                             tag="ccp", name="ccp")
                        for hf in range(2):
                            nc.vector.tensor_tensor(
                                out=ccp0[:, 2 * hf:2 * hf + 2, :, :],
                                in0=av0[:, 2 * hf:2 * hf + 2, :, :],
                                in1=rt[:, 2 * hf:2 * hf + 2, :, :].to_broadcast(
                                    [128, 2, 2, 64]),
                                op=_AluOp.mult,
                            )
                    else:
                        ccp0 = norm_c(av0, den)

                    holder = {}

                    def trans_unit():
                        # 4 full-width PE transposes (qc-major ccp makes each
                        # a [128q,128f] -> [128f,128q] square), then one DVE
                        # copy psum->sbuf (split per half for the final block
                        # so oproj starts off the first half)
                        cc = ccs_pool.tile([128, 512], BF16, tag="cc",
                                           name="cc")
                        for hf in range(2):
                            for qc in (2 * hf, 2 * hf + 1):
                                nc.tensor.transpose(
                                    ct[:, qc * 128:(qc + 1) * 128],
                                    ccp0[:, qc, :, :].rearrange(
                                        "p a b -> p (a b)"),
                                    ident_sb,
                                )
                            if final:
                                nc.vector.tensor_copy(
                                    cc[:, hf * 256:hf * 256 + 256],
                                    ct[:, hf * 256:hf * 256 + 256])
                        if not final:
                            nc.vector.tensor_copy(cc, ct)
                        holder["cc"] = cc

                    def oproj2(n2, half):
                        cc = holder["cc"]
                        for ot in (2 * half, 2 * half + 1):
                            po = mmp.tile([128, 512], F32, tag="po", name="po")
                            nc.tensor.matmul(
                                po,
                                lhsT=cc[:, ot * 128:(ot + 1) * 128],
                                rhs=wo_sb[:, n2 * 512:(n2 + 1) * 512],
                                start=True, stop=True,
                            )
                            ob = ob_pool.tile([128, 512], BF16, tag="ob", name="ob")
                            if act and ot % 2 == 1:
                                nc.scalar.copy(ob, po)
                            else:
                                nc.vector.tensor_copy(ob, po)
                            q = sqs[s_i[0] % len(sqs)]
                            s_i[0] += 1
                            q.dma_start(
                                out=out[tok0 + ot * 128: tok0 + (ot + 1) * 128,
                                        n2 * 512:(n2 + 1) * 512],
                                in_=ob)

                    return [(trans_unit, 500.0, None)] + [
                        (lambda n2=n2, h2=h2: oproj2(n2, h2), 430.0, None)
                        for n2 in range(2) for h2 in range(2)]

                # proj units (~0.85us each), keyed for the drip schedule
                U = {}
                for g2 in range(16):
                    U[f"v{g2}"] = lambda g2=g2: proj_v2(g2)
                for ti in range(4, 8):
                    for hf, hn in ((0, "a"), (1, "b")):
                        U[f"q{ti}{hn}"] = lambda ti=ti, hf=hf: proj_qk_half("q", ti, hf)
                        U[f"k{ti}{hn}"] = lambda ti=ti, hf=hf: proj_qk_half("k", ti, hf)

                # drip allocation: units assigned to a block are guaranteed
                # emitted within it (deadline); the in-block budget logic
                # paces pops so PE work covers each group's exp time.
                # Batch-1 runs ascending so its K/V projections spread across
                # the batch-1 blocks; the final (1,2) block's P2 drains with
                # the scalar engine idle.
                takes = {
                    (0, 3): [(f"v{g}", g + 2) for g in range(7)] + [("v7", 8)],
                    (0, 2): [("q7a", None), ("q7b", None),
                             ("k4a", None), ("k4b", None)],
                    (0, 1): [("q6a", None), ("q6b", None),
                             ("k5a", None), ("k5b", None)],
                    (1, 3): [("v8", 1), ("v9", 2), ("v10", 3),
                             ("k6a", 4), ("k6b", 4), ("v11", 4),
                             ("v12", 5), ("k7a", 6), ("k7b", 6),
                             ("v13", 6), ("v14", 7), ("v15", 8)],
                    (1, 2): [("q5a", None), ("q5b", None)],
                    (0, 0): [],
                    (1, 1): [("q4a", None), ("q4b", None)],
                    (1, 0): [],
                }
                seq = [(0, 3), (0, 2), (0, 1), (1, 3), (1, 2),
                       (0, 0), (1, 1), (1, 0)]
                # the last blocks' stores spread across all three DMA
                # queues: at the drain the sync queue otherwise serializes
                # one store per ~700ns while scalar/gpsimd queues sit idle
                store_qs = {
                    (1, 2): [nc.sync, nc.gpsimd],
                    (1, 1): [nc.sync, nc.gpsimd],
                    (1, 0): [nc.scalar, nc.sync],
                }
                pending = []
                for b, qt in seq:
                    drip = pending + [(U[k], 853.0, due)
                                      for k, due in takes[(b, qt)]]
                    pending = attention(
                        b, qt, drip,
                        act=(b, qt) in [(1, 2), (1, 1), (1, 0)],
                        pop_start=1 if (b, qt) in [(0, 3), (0, 2)] else 0,
                        store_q=store_qs.get((b, qt)),
                        final=(b, qt) == (1, 0))
                for f, _cost, _due in pending:
                    f()
    return nc


_NC_CACHE = None


def _get_nc():
    global _NC_CACHE
    if _NC_CACHE is None:
        _NC_CACHE = build_nc()
        if not _NC_CACHE.is_finalized():
            _NC_CACHE.finalize()
    return _NC_CACHE


def _make_cmask():
    # at layout is [k, q]: valid (unmasked) iff k_rel <= q_rel
    p = np.arange(128)[:, None]
    f = np.arange(128)[None, :]
    return (p <= f).astype(NP_BF16)


def _shard_inputs(x, Wq, bq, Wk, bk, Wv, bv, Wo, bo):
    x = np.asarray(x, np.float32)
    Wq, Wk, Wv, Wo = (np.asarray(a, np.float32) for a in (Wq, Wk, Wv, Wo))
    bq, bk, bv = (np.asarray(a, np.float32) for a in (bq, bk, bv))

    xT = np.ascontiguousarray(x.reshape(T, D_MODEL).T).astype(NP_BF16)
    cmask = _make_cmask()
    ident = np.eye(128, dtype=NP_BF16)

    def wflat(W):
        # [1024, 128] -> [128, 8*128]: element (p, c*128+f) = W[c*128+p, f]
        return np.ascontiguousarray(
            W.reshape(KC, 128, FPC).transpose(1, 0, 2).reshape(128, KC * FPC)
        ).astype(NP_BF16)

    in_maps = []
    for c in range(N_CORES):
        fs = slice(c * FPC, (c + 1) * FPC)
        in_maps.append({
            "xT": xT,
            "wq": wflat(Wq[:, fs] / 8.0),
            "wk": wflat(Wk[:, fs]),
            "wv": wflat(Wv[:, fs]),
            "wo": np.ascontiguousarray(Wo[fs, :]).astype(NP_BF16),
            "bq": np.ascontiguousarray((bq[fs] / 8.0)[:, None]),
            "bk": np.ascontiguousarray(bk[fs][:, None]),
            "bv": np.ascontiguousarray(bv[fs]).reshape(1, FPC),
            "cmask": cmask,
            "ident": ident,
        })
    return in_maps


def _gather(results, bo):
    total = np.zeros((T, D_MODEL), np.float32)
    for c in range(N_CORES):
        total += np.asarray(results[c]["out"], np.float32)
    total += np.asarray(bo, np.float32)[None, :]
    return total.reshape(B, S, D_MODEL)


def kernel(x, Wq, bq, Wk, bk, Wv, bv, Wo, bo):
    in_maps = _shard_inputs(x, Wq, bq, Wk, bk, Wv, bv, Wo, bo)
    nc = _get_nc()
    res = run_bass_kernel_spmd(nc, in_maps, list(range(N_CORES)))
    return _gather(res.results, bo)


if __name__ == "__main__":
    rng = np.random.default_rng(0)
    x = rng.standard_normal((B, S, D_MODEL)).astype(np.float32)
    sc = 1 / np.sqrt(D_MODEL)
    args = dict(
        x=x,
        Wq=rng.standard_normal((D_MODEL, D_MODEL)).astype(np.float32) * sc,
        bq=np.zeros(D_MODEL, np.float32),
        Wk=rng.standard_normal((D_MODEL, D_MODEL)).astype(np.float32) * sc,
        bk=np.zeros(D_MODEL, np.float32),
        Wv=rng.standard_normal((D_MODEL, D_MODEL)).astype(np.float32) * sc,
        bv=np.zeros(D_MODEL, np.float32),
        Wo=rng.standard_normal((D_MODEL, D_MODEL)).astype(np.float32) * sc,
        bo=np.zeros(D_MODEL, np.float32),
    )
    out = kernel(**args)
    print("kernel output", out.shape, out.dtype, np.abs(out).max())


# revision 102
# speedup vs baseline: 1.0039x; 1.0039x over previous
"""Causal multi-head attention on 8 Trainium2 NeuronCores — v3.

Tensor-parallel over heads (16 heads / 8 cores = 2 heads per core).
Each core: full activations x^T (bf16, [d_model, B*S]), column-shard of
Wq/Wk/Wv ([1024, 128], wq pre-scaled by 1/8), row-shard of Wo ([128, 1024]).
Host sums the 8 partial outputs (contraction-sharded output projection).

v3 changes vs v2:
  - cc ([feat, tok]) built by ONE hwdge DMA-transpose per block instead of
    8 PE transposes + a DVE copy; av/ccp switch to qc-major layout
    [128, 4qc, 2h, 64] so the 512-col transpose lands directly in
    [feat, (qc q)] order.
  - den moves to its own PSUM bank ([128, 4, 2] f32).
  - Q bias applied on DVE (scalar engine stays pure exp).
  - Proj drip rescheduled: batch-1 K/V tiles drip INSIDE block (1,3)
    right before the groups that need them, so the PE stays fed through
    the exp-bound batch-1 blocks.
  - First x/wq/wk slices split across both DMA queues for a faster start.
"""

import os
import sys

for p in ("/opt/trn_rl_repo", "/root/.axon_site/_ro/trn_rl_repo"):
    if os.path.isdir(p) and p not in sys.path:
        sys.path.append(p)

import numpy as np
import ml_dtypes

import concourse.bass as bass
import concourse.bacc as bacc
import concourse.mybir as mybir
import concourse.tile as tile
from concourse.bass_utils import run_bass_kernel_spmd

BF16 = mybir.dt.bfloat16
F32 = mybir.dt.float32
NP_BF16 = ml_dtypes.bfloat16

D_MODEL = 1024
NUM_HEADS = 16
D_K = 64
B = 2
S = 2048
T = B * S            # 4096 tokens
N_CORES = 8
FPC = 128            # features per core (2 heads x 64)
N_QT = S // 512      # 4 q-tiles of 512 per batch
N_KT = S // 128      # 16 k-tiles of 128 per batch
KC = D_MODEL // 128  # 8 contraction chunks for the projections

_AluOp = mybir.AluOpType
_Act = mybir.ActivationFunctionType


def build_nc():
    nc = bacc.Bacc()

    xT = nc.declare_dram_parameter("xT", [D_MODEL, T], BF16, isOutput=False)
    wq = nc.declare_dram_parameter("wq", [128, KC * FPC], BF16, isOutput=False)
    wk = nc.declare_dram_parameter("wk", [128, KC * FPC], BF16, isOutput=False)
    wv = nc.declare_dram_parameter("wv", [128, KC * FPC], BF16, isOutput=False)
    wo = nc.declare_dram_parameter("wo", [FPC, D_MODEL], BF16, isOutput=False)
    bq = nc.declare_dram_parameter("bq", [FPC, 1], F32, isOutput=False)
    bk = nc.declare_dram_parameter("bk", [FPC, 1], F32, isOutput=False)
    bv = nc.declare_dram_parameter("bv", [1, FPC], F32, isOutput=False)
    cmask = nc.declare_dram_parameter("cmask", [128, 128], BF16, isOutput=False)
    ident = nc.declare_dram_parameter("ident", [128, 128], BF16, isOutput=False)
    out = nc.declare_dram_parameter("out", [T, D_MODEL], BF16, isOutput=True)

    with tile.TileContext(nc) as tc:
        with (
            tc.tile_pool(name="persist", bufs=1) as persist,
            tc.tile_pool(name="at_pool", bufs=18) as at_pool,
            tc.tile_pool(name="ccp_pool", bufs=4) as ccp_pool,
            tc.tile_pool(name="ccs_pool", bufs=4) as ccs_pool,
            tc.tile_pool(name="rt_pool", bufs=4) as rt_pool,
            tc.tile_pool(name="ob_pool", bufs=8) as ob_pool,
        ):
            # ---------- persistent SBUF ----------

            # The first projection matmuls need wq/wk chunk 0 and
            # xt[0][0][:, ti*512...]; split those leading slices across
            # both DMA queues so the PE can start ~2.5us in.
            xt = [[persist.tile([128, S], BF16, tag=f"xt{c}_{bb}", name=f"xt{c}_{bb}")
                   for bb in range(B)] for c in range(KC)]

            wq_sb = persist.tile([128, KC, FPC], BF16, tag="wq", name="wq")
            wk_sb = persist.tile([128, KC, FPC], BF16, tag="wk", name="wk")
            wv_sb = persist.tile([128, KC, FPC], BF16, tag="wv", name="wv")

            # gpsimd queue: x chunk 0 leading slices; sync queue: wq/wk
            # chunk 0 — the two queues' fixed costs overlap so the first
            # projection matmul can start ~2.6us in
            wq_fl = wq_sb[:, :, :].rearrange("p c f -> p (c f)")
            wk_fl = wk_sb[:, :, :].rearrange("p c f -> p (c f)")
            wv_fl = wv_sb[:, :, :].rearrange("p c f -> p (c f)")
            nc.gpsimd.dma_start(out=xt[0][0][:, 0:512], in_=xT[0:128, 0:512])
            nc.sync.dma_start(out=wq_fl[:, 0:2 * FPC], in_=wq[:, 0:2 * FPC])
            nc.sync.dma_start(out=wk_fl[:, 0:2 * FPC], in_=wk[:, 0:2 * FPC])
            nc.gpsimd.dma_start(out=xt[0][0][:, 512:1024],
                                in_=xT[0:128, 512:1024])
            nc.gpsimd.dma_start(out=xt[0][0][:, 1024:2048],
                                in_=xT[0:128, 1024:2048])
            nc.sync.dma_start(out=wq_fl[:, 2 * FPC:], in_=wq[:, 2 * FPC:])
            nc.sync.dma_start(out=wk_fl[:, 2 * FPC:], in_=wk[:, 2 * FPC:])
            # split b0 chunk loads across both queues: gpsimd's software
            # descriptor-gen chain (~1us per DMA) otherwise serializes
            # chunk availability
            for c in range(1, KC):
                if c in (1, 2):
                    nc.sync.dma_start(
                        out=xt[c][0], in_=xT[c * 128:(c + 1) * 128, 0:S],
                    )
                else:
                    nc.gpsimd.dma_start(
                        out=xt[c][0], in_=xT[c * 128:(c + 1) * 128, 0:S],
                    )

            nc.sync.dma_start(out=wv_fl[:, 0:4 * FPC], in_=wv[:, 0:4 * FPC])
            nc.sync.dma_start(out=wv_fl[:, 4 * FPC:], in_=wv[:, 4 * FPC:])
            wo_sb = persist.tile([128, D_MODEL], BF16, tag="wo")
            nc.sync.dma_start(out=wo_sb, in_=wo[:, :])

            bq_sb = persist.tile([128, 1], F32, tag="bq")
            nc.sync.dma_start(out=bq_sb, in_=bq[:, :])
            bk_sb = persist.tile([128, 1], F32, tag="bk")
            nc.sync.dma_start(out=bk_sb, in_=bk[:, :])
            bv_sb = persist.tile([128, FPC], F32, tag="bv")
            nc.gpsimd.dma_start(out=bv_sb, in_=bv.ap().to_broadcast([128, FPC]))

            oz_sb = persist.tile([128, 2], BF16, tag="oz")
            nc.vector.memset(oz_sb[:, 0:1], 1.0)
            nc.vector.memset(oz_sb[:, 1:2], 0.0)
            mask_sb = persist.tile([128, 128], BF16, tag="cmask")
            nc.sync.dma_start(out=mask_sb, in_=cmask[:, :])
            ident_sb = persist.tile([128, 128], BF16, tag="ident")
            nc.sync.dma_start(out=ident_sb, in_=ident[:, :])

            qt_tiles = [persist.tile([128, 512], BF16, tag=f"qt{i}", name=f"qt{i}")
                        for i in range(T // 512)]
            kt_tiles = [persist.tile([128, 512], BF16, tag=f"kt{i}", name=f"kt{i}")
                        for i in range(T // 512)]
            v_sb = [persist.tile([128, 130], BF16, tag=f"v{g}", name=f"v{g}")
                    for g in range(T // 128)]

            for c in range(KC):
                nc.gpsimd.dma_start(
                    out=xt[c][1], in_=xT[c * 128:(c + 1) * 128, S:2 * S],
                )

            # ---- startup: batch-0 Q/K projections, chunk-outer so the PE
            # starts as soon as x chunk 0 lands (8 psum banks, then freed)
            # ones columns for the AV denominator, one strided memset per
            # v tile, emitted here so the DVE queue is clear at t=0
            for g in range(T // 128):
                nc.vector.memset(
                    v_sb[g][:, :].rearrange("p (a b) -> p a b", a=2)[:, :, 64:65],
                    1.0)

            with tc.tile_pool(name="startup", bufs=1, space="PSUM") as sup:
                pq0 = [sup.tile([128, 512], F32, tag=f"spq{i}", name=f"spq{i}")
                       for i in range(4)]
                pk0 = [sup.tile([128, 512], F32, tag=f"spk{i}", name=f"spk{i}")
                       for i in range(4)]


                def evac_qk(kind, ti):
                    # q on Act, k on DVE: startup evacs split across engines
                    if kind == "q":
                        nc.scalar.add(qt_tiles[ti], pq0[ti], bq_sb[:, 0:1])
                    else:
                        nc.vector.tensor_scalar(
                            out=kt_tiles[ti], in0=pk0[ti], scalar1=bk_sb[:, 0:1],
                            scalar2=None, op0=_AluOp.add,
                        )

                for c in range(KC - 1):
                    for ti in range(4):
                        sl = slice(ti * 512, ti * 512 + 512)
                        nc.tensor.matmul(
                            pq0[ti], lhsT=wq_sb[:, c, :], rhs=xt[c][0][:, sl],
                            start=(c == 0), stop=False,
                        )
                        nc.tensor.matmul(
                            pk0[ti], lhsT=wk_sb[:, c, :], rhs=xt[c][0][:, sl],
                            start=(c == 0), stop=False,
                        )
                c = KC - 1
                for kind, ti in [("k", 0), ("q", 3), ("q", 0), ("q", 1),
                                 ("q", 2), ("k", 1), ("k", 2), ("k", 3)]:
                    sl = slice(ti * 512, ti * 512 + 512)
                    pt, wt = (pq0, wq_sb) if kind == "q" else (pk0, wk_sb)
                    nc.tensor.matmul(
                        pt[ti], lhsT=wt[:, c, :], rhs=xt[c][0][:, sl],
                        start=False, stop=True,
                    )
                    evac_qk(kind, ti)

            with (
                tc.tile_pool(name="scp", bufs=2, space="PSUM") as scp,   # 4 banks
                tc.tile_pool(name="avp", bufs=1, space="PSUM") as avp,   # 1 bank
                tc.tile_pool(name="ctp", bufs=1, space="PSUM") as ctp,   # 1 bank
                tc.tile_pool(name="mmp", bufs=2, space="PSUM") as mmp,   # 2 banks
            ):
                def proj_qk_half(kind, ti, hf):
                    # half a q/k tile (256 tokens): ~0.85us of PE filler
                    bb, loc = ti // 4, (ti % 4) * 512 + hf * 256
                    sl = slice(loc, loc + 256)
                    w_sb, b_sb, dst = ((wq_sb, bq_sb, qt_tiles) if kind == "q"
                                       else (wk_sb, bk_sb, kt_tiles))
                    pp = mmp.tile([128, 512], F32, tag="po", name="pp")
                    for c in range(KC):
                        nc.tensor.matmul(
                            pp[:, 0:256], lhsT=w_sb[:, c, :],
                            rhs=xt[c][bb][:, sl],
                            start=(c == 0), stop=(c == KC - 1),
                        )
                    nc.vector.tensor_scalar(
                        out=dst[ti][:, hf * 256:hf * 256 + 256],
                        in0=pp[:, 0:256], scalar1=b_sb[:, 0:1],
                        scalar2=None, op0=_AluOp.add,
                    )

                def proj_v2(g2):
                    # 2 consecutive 128-token groups: ~0.85us of PE filler
                    bb = g2 // 8
                    pv = mmp.tile([128, 512], F32, tag="po", name="pv")
                    for c in range(KC):
                        for gi in range(2):
                            g = g2 * 2 + gi
                            loc = (g % N_KT) * 128
                            nc.tensor.matmul(
                                pv[:, gi * 128:(gi + 1) * 128],
                                lhsT=xt[c][bb][:, loc:loc + 128],
                                rhs=wv_sb[:, c, :],
                                start=(c == 0 and gi == 0),
                                stop=(c == KC - 1),
                                skip_group_check=True,
                            )
                    for gi in range(2):
                        g = g2 * 2 + gi
                        nc.vector.tensor_tensor(
                            out=v_sb[g][:, :].rearrange("p (h d) -> p h d", h=2)[:, :, 0:64],
                            in0=pv[:, gi * 128:(gi + 1) * 128].rearrange(
                                "p (h d) -> p h d", h=2),
                            in1=bv_sb[:, :].rearrange("p (h d) -> p h d", h=2),
                            op=_AluOp.add,
                        )

                def score_exp(b, qt, h, grp):
                    """Scores + exp + corner masks for one 2-k-tile group.
                    Returns (at_tile, rels)."""
                    hsl = slice(h * 64, (h + 1) * 64)
                    sc = scp.tile([128, 1024], F32, tag="sc", name="sc")
                    at = at_pool.tile([128, 1024], BF16, tag="at", name="at")
                    rels = []
                    for j in range(2):
                        ki = grp * 2 + j
                        rel = ki * 128 - qt * 512
                        r = rel // 128 if rel >= 0 else -1
                        rels.append(r)
                        kt_i = b * 4 + ki // 4
                        ko = (ki % 4) * 128
                        q0 = r * 128 if r > 0 else 0
                        nc.tensor.matmul(
                            sc[:, j * 512 + q0:(j + 1) * 512],
                            lhsT=kt_tiles[kt_i][hsl, ko:ko + 128],
                            rhs=qt_tiles[b * 4 + qt][hsl, q0:512],
                            start=True, stop=True,
                        )
                    if rels[1] <= 0:
                        nc.scalar.activation(out=at, in_=sc, func=_Act.Exp)
                    elif rels[0] == 0:
                        # one exp across both ranges; cols [512,640) are
                        # garbage but never read downstream
                        nc.scalar.activation(out=at, in_=sc, func=_Act.Exp)
                    else:
                        q0a = rels[0] * 128 if rels[0] > 0 else 0
                        nc.scalar.activation(
                            out=at[:, q0a:512], in_=sc[:, q0a:512],
                            func=_Act.Exp)
                        q0b = 512 + rels[1] * 128
                        nc.scalar.activation(
                            out=at[:, q0b:1024], in_=sc[:, q0b:1024],
                            func=_Act.Exp)
                    for j in range(2):
                        r = rels[j]
                        if r >= 0:
                            c0 = j * 512 + r * 128
                            nc.vector.tensor_tensor(
                                out=at[:, c0:c0 + 128],
                                in0=at[:, c0:c0 + 128],
                                in1=mask_sb,
                                op=_AluOp.mult,
                            )
                    return at, rels

                def av_group(b, h, av, den, at, rels, grp):
                    for j in range(2):
                        ki = grp * 2 + j
                        r = rels[j]
                        g = b * N_KT + ki
                        for qc in range(max(r, 0), 4):
                            sl = slice(j * 512 + qc * 128,
                                       j * 512 + (qc + 1) * 128)
                            nc.tensor.matmul(
                                av[:, qc, h, :],
                                lhsT=at[:, sl],
                                rhs=v_sb[g][:, h * 65:h * 65 + 64],
                                start=(h == 0 and ki == 0 and qc == 0),
                                stop=(r >= 0 and qc == r),
                                skip_group_check=True,
                            )
                            nc.tensor.matmul(
                                den[:, qc, h:h + 1],
                                lhsT=at[:, sl],
                                rhs=oz_sb[:, 0:1],
                                start=False,
                                stop=(r >= 0 and qc == r),
                                skip_group_check=True,
                            )

                def norm_c(av, den):
                    rt = rt_pool.tile([128, 4, 2, 1], F32, tag="rt", name="rt")
                    nc.vector.reciprocal(rt, den[:, :, :, None])
                    ccp = ccp_pool.tile([128, 4, 2, 64], BF16, tag="ccp",
                                        name="ccp")
                    nc.vector.tensor_tensor(
                        out=ccp, in0=av,
                        in1=rt[:, :, :, :].to_broadcast([128, 4, 2, 64]),
                        op=_AluOp.mult,
                    )
                    return ccp

                def attention(b, qt, fillers, act=False, pop_start=0,
                              store_q=None, final=False):
                    """One (batch, 512-q-tile) attention block, both heads,
                    h0/h1 score+exp chains interleaved.  Fillers are
                    (callable, due_iter|None); units due by iteration g are
                    emitted right after iteration g-1's score matmuls so the
                    tiles they write are ready for g's reads.  Returns
                    deferred units: [dma-transpose, oproj x4] to drip into
                    the next block."""
                    tok0 = b * S + qt * 512
                    sqs = store_q or [nc.sync]
                    s_i = [0]
                    nk = (qt + 1) * 4          # visible 128-k-tiles
                    ng = nk // 2
                    av0 = avp.tile([128, 4, 2, 64], F32, tag="av", name="av0")
                    ct528 = ctp.tile([128, 528], BF16, tag="ct", name="ct")
                    ct = ct528[:, 0:512]
                    den = ct528[:, 512:528].bitcast(F32).rearrange(
                        "p (a b) -> p a b", a=4)

                    def run_due(limit):
                        popped = 0.0
                        i = 0
                        while i < len(fillers):
                            f, cost, due = fillers[i]
                            if due is not None and due <= limit:
                                fillers.pop(i)
                                f()
                                popped += cost
                            else:
                                i += 1
                        return popped

                    def est(rels):
                        # (act_ns, pe_ns) for one group's two heads
                        cols = sum(512 - max(r, 0) * 128 for r in rels)
                        ninst = 2 if (rels[0] > 0 and rels[1] > 0) else 1
                        act = 2 * (0.833 * cols + 185 * ninst)
                        pe = 0.8333 * cols          # scores, both heads
                        return act, pe

                    av_pe = 0.0                      # AV cols of prev group
                    deficit = 0.0
                    prev = None
                    for grp in range(ng):
                        run_due(grp)  # safety: anything this grp's reads need
                        at0, rels0 = score_exp(b, qt, 0, grp)
                        at1, rels1 = score_exp(b, qt, 1, grp)
                        a_ns, p_ns = est(rels0)
                        deficit += a_ns - p_ns - av_pe
                        av_pe = 2 * 0.4167 * 65 * sum(
                            4 - max(r, 0) for r in rels0)
                        deficit -= run_due(grp + 1)
                        if grp < pop_start:
                            deficit = min(deficit, 0.0)
                        while deficit > -300 and fillers:
                            f, cost, _due = fillers.pop(0)
                            f()
                            deficit -= cost
                        if grp == min(1, ng - 1):
                            # zero-init den col 0; start=True resets the
                            # bank's accumulation-group tracking.  Emitted as
                            # late as possible so it doesn't stall the PE on
                            # the previous block's ct-bank read.
                            dfl = den[:, :, :].rearrange("p a b -> p (a b)")
                            nc.tensor.matmul(
                                dfl[:, 0:1], lhsT=mask_sb, rhs=oz_sb[:, 1:2],
                                start=True, stop=False, skip_group_check=True,
                            )
                        # AV lags one group so it never waits on its own exp
                        if prev is not None:
                            av_group(b, 0, av0, den, prev[0][0], prev[0][1], grp - 1)
                            av_group(b, 1, av0, den, prev[1][0], prev[1][1], grp - 1)
                        prev = ((at0, rels0), (at1, rels1))
                    av_group(b, 0, av0, den, prev[0][0], prev[0][1], ng - 1)
                    av_group(b, 1, av0, den, prev[1][0], prev[1][1], ng - 1)
                    for f, _cost, _due in fillers:
                        f()
                    if final:
                        # split norm so the first transpose pair starts as
                        # soon as the first half of ccp is normalized
                        rt = rt_pool.tile([128, 4, 2, 1], F32, tag="rt",
                                          name="rt")
                        nc.vector.reciprocal(rt, den[:, :, :, None])
                        ccp0 = ccp_pool.tile([128, 4, 2, 64], BF16,
                                             tag="ccp", name="ccp")
                        for hf in range(2):
                            nc.vector.tensor_tensor(
                                out=ccp0[:, 2 * hf:2 * hf + 2, :, :],
                                in0=av0[:, 2 * hf:2 * hf + 2, :, :],
                                in1=rt[:, 2 * hf:2 * hf + 2, :, :].to_broadcast(
                                    [128, 2, 2, 64]),
                                op=_AluOp.mult,
                            )
                    else:
                        ccp0 = norm_c(av0, den)

                    holder = {}

                    def trans_unit():
                        # 4 full-width PE transposes (qc-major ccp makes each
                        # a [128q,128f] -> [128f,128q] square), then one DVE
                        # copy psum->sbuf (split per half for the final block
                        # so oproj starts off the first half)
                        cc = ccs_pool.tile([128, 512], BF16, tag="cc",
                                           name="cc")
                        for hf in range(2):
                            for qc in (2 * hf, 2 * hf + 1):
                                nc.tensor.transpose(
                                    ct[:, qc * 128:(qc + 1) * 128],
                                    ccp0[:, qc, :, :].rearrange(
                                        "p a b -> p (a b)"),
                                    ident_sb,
                                )
                            if final:
                                nc.vector.tensor_copy(
                                    cc[:, hf * 256:hf * 256 + 256],
                                    ct[:, hf * 256:hf * 256 + 256])
                        if not final:
                            nc.vector.tensor_copy(cc, ct)
                        holder["cc"] = cc

                    def oproj2(n2, half):
                        cc = holder["cc"]
                        for ot in (2 * half, 2 * half + 1):
                            po = mmp.tile([128, 512], F32, tag="po", name="po")
                            nc.tensor.matmul(
                                po,
                                lhsT=cc[:, ot * 128:(ot + 1) * 128],
                                rhs=wo_sb[:, n2 * 512:(n2 + 1) * 512],
                                start=True, stop=True,
                            )
                            ob = ob_pool.tile([128, 512], BF16, tag="ob", name="ob")
                            if act and ot % 2 == 1:
                                nc.scalar.copy(ob, po)
                            else:
                                nc.vector.tensor_copy(ob, po)
                            q = sqs[s_i[0] % len(sqs)]
                            s_i[0] += 1
                            q.dma_start(
                                out=out[tok0 + ot * 128: tok0 + (ot + 1) * 128,
                                        n2 * 512:(n2 + 1) * 512],
                                in_=ob)

                    return [(trans_unit, 500.0, None)] + [
                        (lambda n2=n2, h2=h2: oproj2(n2, h2), 430.0, None)
                        for n2 in range(2) for h2 in range(2)]

                # proj units (~0.85us each), keyed for the drip schedule
                U = {}
                for g2 in range(16):
                    U[f"v{g2}"] = lambda g2=g2: proj_v2(g2)
                for ti in range(4, 8):
                    for hf, hn in ((0, "a"), (1, "b")):
                        U[f"q{ti}{hn}"] = lambda ti=ti, hf=hf: proj_qk_half("q", ti, hf)
                        U[f"k{ti}{hn}"] = lambda ti=ti, hf=hf: proj_qk_half("k", ti, hf)

                # drip allocation: units assigned to a block are guaranteed
                # emitted within it (deadline); the in-block budget logic
                # paces pops so PE work covers each group's exp time.
                # Batch-1 runs ascending so its K/V projections spread across
                # the batch-1 blocks; the final (1,2) block's P2 drains with
                # the scalar engine idle.
                takes = {
                    (0, 3): [(f"v{g}", g + 2) for g in range(7)] + [("v7", 8)],
                    (0, 2): [("q7a", None), ("q7b", None),
                             ("k4a", None), ("k4b", None)],
                    (0, 1): [("q6a", None), ("q6b", None),
                             ("k5a", None), ("k5b", None)],
                    (1, 3): [("v8", 1), ("v9", 2), ("v10", 3),
                             ("k6a", 4), ("k6b", 4), ("v11", 4),
                             ("v12", 5), ("k7a", 6), ("k7b", 6),
                             ("v13", 6), ("v14", 7), ("v15", 8)],
                    (1, 2): [("q5a", None), ("q5b", None)],
                    (0, 0): [],
                    (1, 1): [("q4a", None), ("q4b", None)],
                    (1, 0): [],
                }
                seq = [(0, 3), (0, 2), (0, 1), (1, 3), (1, 2),
                       (0, 0), (1, 1), (1, 0)]
                # the last blocks' stores spread across all three DMA
                # queues: at the drain the sync queue otherwise serializes
                # one store per ~700ns while scalar/gpsimd queues sit idle
                store_qs = {
                    (1, 2): [nc.sync, nc.gpsimd],
                    (1, 1): [nc.sync, nc.gpsimd],
                    (1, 0): [nc.scalar, nc.sync],
                }
                pending = []
                for b, qt in seq:
                    drip = pending + [(U[k], 853.0, due)
                                      for k, due in takes[(b, qt)]]
                    pending = attention(
                        b, qt, drip,
                        act=(b, qt) in [(1, 2), (1, 1), (1, 0)],
                        pop_start=1 if (b, qt) in [(0, 3), (0, 2)] else 0,
                        store_q=store_qs.get((b, qt)),
                        final=(b, qt) in [(1, 1), (1, 0)])
                for f, _cost, _due in pending:
                    f()
    return nc


_NC_CACHE = None


def _get_nc():
    global _NC_CACHE
    if _NC_CACHE is None:
        _NC_CACHE = build_nc()
        if not _NC_CACHE.is_finalized():
            _NC_CACHE.finalize()
    return _NC_CACHE


def _make_cmask():
    # at layout is [k, q]: valid (unmasked) iff k_rel <= q_rel
    p = np.arange(128)[:, None]
    f = np.arange(128)[None, :]
    return (p <= f).astype(NP_BF16)


def _shard_inputs(x, Wq, bq, Wk, bk, Wv, bv, Wo, bo):
    x = np.asarray(x, np.float32)
    Wq, Wk, Wv, Wo = (np.asarray(a, np.float32) for a in (Wq, Wk, Wv, Wo))
    bq, bk, bv = (np.asarray(a, np.float32) for a in (bq, bk, bv))

    xT = np.ascontiguousarray(x.reshape(T, D_MODEL).T).astype(NP_BF16)
    cmask = _make_cmask()
    ident = np.eye(128, dtype=NP_BF16)

    def wflat(W):
        # [1024, 128] -> [128, 8*128]: element (p, c*128+f) = W[c*128+p, f]
        return np.ascontiguousarray(
            W.reshape(KC, 128, FPC).transpose(1, 0, 2).reshape(128, KC * FPC)
        ).astype(NP_BF16)

    in_maps = []
    for c in range(N_CORES):
        fs = slice(c * FPC, (c + 1) * FPC)
        in_maps.append({
            "xT": xT,
            "wq": wflat(Wq[:, fs] / 8.0),
            "wk": wflat(Wk[:, fs]),
            "wv": wflat(Wv[:, fs]),
            "wo": np.ascontiguousarray(Wo[fs, :]).astype(NP_BF16),
            "bq": np.ascontiguousarray((bq[fs] / 8.0)[:, None]),
            "bk": np.ascontiguousarray(bk[fs][:, None]),
            "bv": np.ascontiguousarray(bv[fs]).reshape(1, FPC),
            "cmask": cmask,
            "ident": ident,
        })
    return in_maps


def _gather(results, bo):
    total = np.zeros((T, D_MODEL), np.float32)
    for c in range(N_CORES):
        total += np.asarray(results[c]["out"], np.float32)
    total += np.asarray(bo, np.float32)[None, :]
    return total.reshape(B, S, D_MODEL)


def kernel(x, Wq, bq, Wk, bk, Wv, bv, Wo, bo):
    in_maps = _shard_inputs(x, Wq, bq, Wk, bk, Wv, bv, Wo, bo)
    nc = _get_nc()
    res = run_bass_kernel_spmd(nc, in_maps, list(range(N_CORES)))
    return _gather(res.results, bo)


if __name__ == "__main__":
    rng = np.random.default_rng(0)
    x = rng.standard_normal((B, S, D_MODEL)).astype(np.float32)
    sc = 1 / np.sqrt(D_MODEL)
    args = dict(
        x=x,
        Wq=rng.standard_normal((D_MODEL, D_MODEL)).astype(np.float32) * sc,
        bq=np.zeros(D_MODEL, np.float32),
        Wk=rng.standard_normal((D_MODEL, D_MODEL)).astype(np.float32) * sc,
        bk=np.zeros(D_MODEL, np.float32),
        Wv=rng.standard_normal((D_MODEL, D_MODEL)).astype(np.float32) * sc,
        bv=np.zeros(D_MODEL, np.float32),
        Wo=rng.standard_normal((D_MODEL, D_MODEL)).astype(np.float32) * sc,
        bo=np.zeros(D_MODEL, np.float32),
    )
    out = kernel(**args)
    print("kernel output", out.shape, out.dtype, np.abs(out).max())


# revision 103
# speedup vs baseline: 1.0049x; 1.0009x over previous
"""Causal multi-head attention on 8 Trainium2 NeuronCores — v3.

Tensor-parallel over heads (16 heads / 8 cores = 2 heads per core).
Each core: full activations x^T (bf16, [d_model, B*S]), column-shard of
Wq/Wk/Wv ([1024, 128], wq pre-scaled by 1/8), row-shard of Wo ([128, 1024]).
Host sums the 8 partial outputs (contraction-sharded output projection).

v3 changes vs v2:
  - cc ([feat, tok]) built by ONE hwdge DMA-transpose per block instead of
    8 PE transposes + a DVE copy; av/ccp switch to qc-major layout
    [128, 4qc, 2h, 64] so the 512-col transpose lands directly in
    [feat, (qc q)] order.
  - den moves to its own PSUM bank ([128, 4, 2] f32).
  - Q bias applied on DVE (scalar engine stays pure exp).
  - Proj drip rescheduled: batch-1 K/V tiles drip INSIDE block (1,3)
    right before the groups that need them, so the PE stays fed through
    the exp-bound batch-1 blocks.
  - First x/wq/wk slices split across both DMA queues for a faster start.
"""

import os
import sys

for p in ("/opt/trn_rl_repo", "/root/.axon_site/_ro/trn_rl_repo"):
    if os.path.isdir(p) and p not in sys.path:
        sys.path.append(p)

import numpy as np
import ml_dtypes

import concourse.bass as bass
import concourse.bacc as bacc
import concourse.mybir as mybir
import concourse.tile as tile
from concourse.bass_utils import run_bass_kernel_spmd

BF16 = mybir.dt.bfloat16
F32 = mybir.dt.float32
NP_BF16 = ml_dtypes.bfloat16

D_MODEL = 1024
NUM_HEADS = 16
D_K = 64
B = 2
S = 2048
T = B * S            # 4096 tokens
N_CORES = 8
FPC = 128            # features per core (2 heads x 64)
N_QT = S // 512      # 4 q-tiles of 512 per batch
N_KT = S // 128      # 16 k-tiles of 128 per batch
KC = D_MODEL // 128  # 8 contraction chunks for the projections

_AluOp = mybir.AluOpType
_Act = mybir.ActivationFunctionType


def build_nc():
    nc = bacc.Bacc()

    xT = nc.declare_dram_parameter("xT", [D_MODEL, T], BF16, isOutput=False)
    wq = nc.declare_dram_parameter("wq", [128, KC * FPC], BF16, isOutput=False)
    wk = nc.declare_dram_parameter("wk", [128, KC * FPC], BF16, isOutput=False)
    wv = nc.declare_dram_parameter("wv", [128, KC * FPC], BF16, isOutput=False)
    wo = nc.declare_dram_parameter("wo", [FPC, D_MODEL], BF16, isOutput=False)
    bq = nc.declare_dram_parameter("bq", [FPC, 1], F32, isOutput=False)
    bk = nc.declare_dram_parameter("bk", [FPC, 1], F32, isOutput=False)
    bv = nc.declare_dram_parameter("bv", [1, FPC], F32, isOutput=False)
    cmask = nc.declare_dram_parameter("cmask", [128, 128], BF16, isOutput=False)
    ident = nc.declare_dram_parameter("ident", [128, 128], BF16, isOutput=False)
    out = nc.declare_dram_parameter("out", [T, D_MODEL], BF16, isOutput=True)

    with tile.TileContext(nc) as tc:
        with (
            tc.tile_pool(name="persist", bufs=1) as persist,
            tc.tile_pool(name="at_pool", bufs=18) as at_pool,
            tc.tile_pool(name="ccp_pool", bufs=4) as ccp_pool,
            tc.tile_pool(name="ccs_pool", bufs=4) as ccs_pool,
            tc.tile_pool(name="rt_pool", bufs=4) as rt_pool,
            tc.tile_pool(name="ob_pool", bufs=8) as ob_pool,
        ):
            # ---------- persistent SBUF ----------

            # The first projection matmuls need wq/wk chunk 0 and
            # xt[0][0][:, ti*512...]; split those leading slices across
            # both DMA queues so the PE can start ~2.5us in.
            xt = [[persist.tile([128, S], BF16, tag=f"xt{c}_{bb}", name=f"xt{c}_{bb}")
                   for bb in range(B)] for c in range(KC)]

            wq_sb = persist.tile([128, KC, FPC], BF16, tag="wq", name="wq")
            wk_sb = persist.tile([128, KC, FPC], BF16, tag="wk", name="wk")
            wv_sb = persist.tile([128, KC, FPC], BF16, tag="wv", name="wv")

            # gpsimd queue: x chunk 0 leading slices; sync queue: wq/wk
            # chunk 0 — the two queues' fixed costs overlap so the first
            # projection matmul can start ~2.6us in
            wq_fl = wq_sb[:, :, :].rearrange("p c f -> p (c f)")
            wk_fl = wk_sb[:, :, :].rearrange("p c f -> p (c f)")
            wv_fl = wv_sb[:, :, :].rearrange("p c f -> p (c f)")
            nc.gpsimd.dma_start(out=xt[0][0][:, 0:512], in_=xT[0:128, 0:512])
            nc.sync.dma_start(out=wq_fl[:, 0:2 * FPC], in_=wq[:, 0:2 * FPC])
            nc.sync.dma_start(out=wk_fl[:, 0:2 * FPC], in_=wk[:, 0:2 * FPC])
            nc.gpsimd.dma_start(out=xt[0][0][:, 512:1024],
                                in_=xT[0:128, 512:1024])
            nc.gpsimd.dma_start(out=xt[0][0][:, 1024:2048],
                                in_=xT[0:128, 1024:2048])
            nc.sync.dma_start(out=wq_fl[:, 2 * FPC:], in_=wq[:, 2 * FPC:])
            nc.sync.dma_start(out=wk_fl[:, 2 * FPC:], in_=wk[:, 2 * FPC:])
            # split b0 chunk loads across both queues: gpsimd's software
            # descriptor-gen chain (~1us per DMA) otherwise serializes
            # chunk availability
            for c in range(1, KC):
                if c in (1, 2):
                    nc.sync.dma_start(
                        out=xt[c][0], in_=xT[c * 128:(c + 1) * 128, 0:S],
                    )
                else:
                    nc.gpsimd.dma_start(
                        out=xt[c][0], in_=xT[c * 128:(c + 1) * 128, 0:S],
                    )

            nc.sync.dma_start(out=wv_fl[:, 0:4 * FPC], in_=wv[:, 0:4 * FPC])
            nc.sync.dma_start(out=wv_fl[:, 4 * FPC:], in_=wv[:, 4 * FPC:])
            wo_sb = persist.tile([128, D_MODEL], BF16, tag="wo")
            nc.sync.dma_start(out=wo_sb, in_=wo[:, :])

            bq_sb = persist.tile([128, 1], F32, tag="bq")
            nc.sync.dma_start(out=bq_sb, in_=bq[:, :])
            bk_sb = persist.tile([128, 1], F32, tag="bk")
            nc.sync.dma_start(out=bk_sb, in_=bk[:, :])
            bv_sb = persist.tile([128, FPC], F32, tag="bv")
            nc.gpsimd.dma_start(out=bv_sb, in_=bv.ap().to_broadcast([128, FPC]))

            oz_sb = persist.tile([128, 2], BF16, tag="oz")
            nc.vector.memset(oz_sb[:, 0:1], 1.0)
            nc.vector.memset(oz_sb[:, 1:2], 0.0)
            mask_sb = persist.tile([128, 128], BF16, tag="cmask")
            nc.sync.dma_start(out=mask_sb, in_=cmask[:, :])
            ident_sb = persist.tile([128, 128], BF16, tag="ident")
            nc.sync.dma_start(out=ident_sb, in_=ident[:, :])

            qt_tiles = [persist.tile([128, 512], BF16, tag=f"qt{i}", name=f"qt{i}")
                        for i in range(T // 512)]
            kt_tiles = [persist.tile([128, 512], BF16, tag=f"kt{i}", name=f"kt{i}")
                        for i in range(T // 512)]
            v_sb = [persist.tile([128, 130], BF16, tag=f"v{g}", name=f"v{g}")
                    for g in range(T // 128)]

            for c in range(KC):
                nc.gpsimd.dma_start(
                    out=xt[c][1], in_=xT[c * 128:(c + 1) * 128, S:2 * S],
                )

            # ---- startup: batch-0 Q/K projections, chunk-outer so the PE
            # starts as soon as x chunk 0 lands (8 psum banks, then freed)
            # ones columns for the AV denominator, one strided memset per
            # v tile, emitted here so the DVE queue is clear at t=0
            for g in range(T // 128):
                nc.vector.memset(
                    v_sb[g][:, :].rearrange("p (a b) -> p a b", a=2)[:, :, 64:65],
                    1.0)

            with tc.tile_pool(name="startup", bufs=1, space="PSUM") as sup:
                pq0 = [sup.tile([128, 512], F32, tag=f"spq{i}", name=f"spq{i}")
                       for i in range(4)]
                pk0 = [sup.tile([128, 512], F32, tag=f"spk{i}", name=f"spk{i}")
                       for i in range(4)]


                def evac_qk(kind, ti):
                    # q on Act, k on DVE: startup evacs split across engines
                    if kind == "q":
                        nc.scalar.add(qt_tiles[ti], pq0[ti], bq_sb[:, 0:1])
                    else:
                        nc.vector.tensor_scalar(
                            out=kt_tiles[ti], in0=pk0[ti], scalar1=bk_sb[:, 0:1],
                            scalar2=None, op0=_AluOp.add,
                        )

                for c in range(KC - 1):
                    for ti in range(4):
                        sl = slice(ti * 512, ti * 512 + 512)
                        nc.tensor.matmul(
                            pq0[ti], lhsT=wq_sb[:, c, :], rhs=xt[c][0][:, sl],
                            start=(c == 0), stop=False,
                        )
                        nc.tensor.matmul(
                            pk0[ti], lhsT=wk_sb[:, c, :], rhs=xt[c][0][:, sl],
                            start=(c == 0), stop=False,
                        )
                c = KC - 1
                for kind, ti in [("k", 0), ("q", 3), ("q", 0), ("q", 1),
                                 ("q", 2), ("k", 1), ("k", 2), ("k", 3)]:
                    sl = slice(ti * 512, ti * 512 + 512)
                    pt, wt = (pq0, wq_sb) if kind == "q" else (pk0, wk_sb)
                    nc.tensor.matmul(
                        pt[ti], lhsT=wt[:, c, :], rhs=xt[c][0][:, sl],
                        start=False, stop=True,
                    )
                    evac_qk(kind, ti)

            with (
                tc.tile_pool(name="scp", bufs=2, space="PSUM") as scp,   # 4 banks
                tc.tile_pool(name="avp", bufs=1, space="PSUM") as avp,   # 1 bank
                tc.tile_pool(name="ctp", bufs=1, space="PSUM") as ctp,   # 1 bank
                tc.tile_pool(name="mmp", bufs=2, space="PSUM") as mmp,   # 2 banks
            ):
                def proj_qk_half(kind, ti, hf):
                    # half a q/k tile (256 tokens): ~0.85us of PE filler
                    bb, loc = ti // 4, (ti % 4) * 512 + hf * 256
                    sl = slice(loc, loc + 256)
                    w_sb, b_sb, dst = ((wq_sb, bq_sb, qt_tiles) if kind == "q"
                                       else (wk_sb, bk_sb, kt_tiles))
                    pp = mmp.tile([128, 512], F32, tag="po", name="pp")
                    for c in range(KC):
                        nc.tensor.matmul(
                            pp[:, 0:256], lhsT=w_sb[:, c, :],
                            rhs=xt[c][bb][:, sl],
                            start=(c == 0), stop=(c == KC - 1),
                        )
                    nc.vector.tensor_scalar(
                        out=dst[ti][:, hf * 256:hf * 256 + 256],
                        in0=pp[:, 0:256], scalar1=b_sb[:, 0:1],
                        scalar2=None, op0=_AluOp.add,
                    )

                def proj_v2(g2):
                    # 2 consecutive 128-token groups: ~0.85us of PE filler
                    bb = g2 // 8
                    pv = mmp.tile([128, 512], F32, tag="po", name="pv")
                    for c in range(KC):
                        for gi in range(2):
                            g = g2 * 2 + gi
                            loc = (g % N_KT) * 128
                            nc.tensor.matmul(
                                pv[:, gi * 128:(gi + 1) * 128],
                                lhsT=xt[c][bb][:, loc:loc + 128],
                                rhs=wv_sb[:, c, :],
                                start=(c == 0 and gi == 0),
                                stop=(c == KC - 1),
                                skip_group_check=True,
                            )
                    for gi in range(2):
                        g = g2 * 2 + gi
                        nc.vector.tensor_tensor(
                            out=v_sb[g][:, :].rearrange("p (h d) -> p h d", h=2)[:, :, 0:64],
                            in0=pv[:, gi * 128:(gi + 1) * 128].rearrange(
                                "p (h d) -> p h d", h=2),
                            in1=bv_sb[:, :].rearrange("p (h d) -> p h d", h=2),
                            op=_AluOp.add,
                        )

                def score_exp(b, qt, h, grp):
                    """Scores + exp + corner masks for one 2-k-tile group.
                    Returns (at_tile, rels)."""
                    hsl = slice(h * 64, (h + 1) * 64)
                    sc = scp.tile([128, 1024], F32, tag="sc", name="sc")
                    at = at_pool.tile([128, 1024], BF16, tag="at", name="at")
                    rels = []
                    for j in range(2):
                        ki = grp * 2 + j
                        rel = ki * 128 - qt * 512
                        r = rel // 128 if rel >= 0 else -1
                        rels.append(r)
                        kt_i = b * 4 + ki // 4
                        ko = (ki % 4) * 128
                        q0 = r * 128 if r > 0 else 0
                        nc.tensor.matmul(
                            sc[:, j * 512 + q0:(j + 1) * 512],
                            lhsT=kt_tiles[kt_i][hsl, ko:ko + 128],
                            rhs=qt_tiles[b * 4 + qt][hsl, q0:512],
                            start=True, stop=True,
                        )
                    if rels[1] <= 0:
                        nc.scalar.activation(out=at, in_=sc, func=_Act.Exp)
                    elif rels[0] == 0:
                        # one exp across both ranges; cols [512,640) are
                        # garbage but never read downstream
                        nc.scalar.activation(out=at, in_=sc, func=_Act.Exp)
                    else:
                        q0a = rels[0] * 128 if rels[0] > 0 else 0
                        nc.scalar.activation(
                            out=at[:, q0a:512], in_=sc[:, q0a:512],
                            func=_Act.Exp)
                        q0b = 512 + rels[1] * 128
                        nc.scalar.activation(
                            out=at[:, q0b:1024], in_=sc[:, q0b:1024],
                            func=_Act.Exp)
                    for j in range(2):
                        r = rels[j]
                        if r >= 0:
                            c0 = j * 512 + r * 128
                            nc.vector.tensor_tensor(
                                out=at[:, c0:c0 + 128],
                                in0=at[:, c0:c0 + 128],
                                in1=mask_sb,
                                op=_AluOp.mult,
                            )
                    return at, rels

                def av_group(b, h, av, den, at, rels, grp):
                    for j in range(2):
                        ki = grp * 2 + j
                        r = rels[j]
                        g = b * N_KT + ki
                        for qc in range(max(r, 0), 4):
                            sl = slice(j * 512 + qc * 128,
                                       j * 512 + (qc + 1) * 128)
                            nc.tensor.matmul(
                                av[:, qc, h, :],
                                lhsT=at[:, sl],
                                rhs=v_sb[g][:, h * 65:h * 65 + 64],
                                start=(h == 0 and ki == 0 and qc == 0),
                                stop=(r >= 0 and qc == r),
                                skip_group_check=True,
                            )
                            nc.tensor.matmul(
                                den[:, qc, h:h + 1],
                                lhsT=at[:, sl],
                                rhs=oz_sb[:, 0:1],
                                start=False,
                                stop=(r >= 0 and qc == r),
                                skip_group_check=True,
                            )

                def norm_c(av, den):
                    rt = rt_pool.tile([128, 4, 2, 1], F32, tag="rt", name="rt")
                    nc.vector.reciprocal(rt, den[:, :, :, None])
                    ccp = ccp_pool.tile([128, 4, 2, 64], BF16, tag="ccp",
                                        name="ccp")
                    nc.vector.tensor_tensor(
                        out=ccp, in0=av,
                        in1=rt[:, :, :, :].to_broadcast([128, 4, 2, 64]),
                        op=_AluOp.mult,
                    )
                    return ccp

                def attention(b, qt, fillers, act=False, pop_start=0,
                              store_q=None, final=False):
                    """One (batch, 512-q-tile) attention block, both heads,
                    h0/h1 score+exp chains interleaved.  Fillers are
                    (callable, due_iter|None); units due by iteration g are
                    emitted right after iteration g-1's score matmuls so the
                    tiles they write are ready for g's reads.  Returns
                    deferred units: [dma-transpose, oproj x4] to drip into
                    the next block."""
                    tok0 = b * S + qt * 512
                    sqs = store_q or [nc.sync]
                    s_i = [0]
                    nk = (qt + 1) * 4          # visible 128-k-tiles
                    ng = nk // 2
                    av0 = avp.tile([128, 4, 2, 64], F32, tag="av", name="av0")
                    ct528 = ctp.tile([128, 528], BF16, tag="ct", name="ct")
                    ct = ct528[:, 0:512]
                    den = ct528[:, 512:528].bitcast(F32).rearrange(
                        "p (a b) -> p a b", a=4)

                    def run_due(limit):
                        popped = 0.0
                        i = 0
                        while i < len(fillers):
                            f, cost, due = fillers[i]
                            if due is not None and due <= limit:
                                fillers.pop(i)
                                f()
                                popped += cost
                            else:
                                i += 1
                        return popped

                    def est(rels):
                        # (act_ns, pe_ns) for one group's two heads
                        cols = sum(512 - max(r, 0) * 128 for r in rels)
                        ninst = 2 if (rels[0] > 0 and rels[1] > 0) else 1
                        act = 2 * (0.833 * cols + 185 * ninst)
                        pe = 0.8333 * cols          # scores, both heads
                        return act, pe

                    av_pe = 0.0                      # AV cols of prev group
                    deficit = 0.0
                    prev = None
                    for grp in range(ng):
                        run_due(grp)  # safety: anything this grp's reads need
                        at0, rels0 = score_exp(b, qt, 0, grp)
                        at1, rels1 = score_exp(b, qt, 1, grp)
                        a_ns, p_ns = est(rels0)
                        deficit += a_ns - p_ns - av_pe
                        av_pe = 2 * 0.4167 * 65 * sum(
                            4 - max(r, 0) for r in rels0)
                        deficit -= run_due(grp + 1)
                        if grp < pop_start:
                            deficit = min(deficit, 0.0)
                        while deficit > -300 and fillers:
                            f, cost, _due = fillers.pop(0)
                            f()
                            deficit -= cost
                        if grp == min(1, ng - 1):
                            # zero-init den col 0; start=True resets the
                            # bank's accumulation-group tracking.  Emitted as
                            # late as possible so it doesn't stall the PE on
                            # the previous block's ct-bank read.
                            dfl = den[:, :, :].rearrange("p a b -> p (a b)")
                            nc.tensor.matmul(
                                dfl[:, 0:1], lhsT=mask_sb, rhs=oz_sb[:, 1:2],
                                start=True, stop=False, skip_group_check=True,
                            )
                        # AV lags one group so it never waits on its own exp
                        if prev is not None:
                            av_group(b, 0, av0, den, prev[0][0], prev[0][1], grp - 1)
                            av_group(b, 1, av0, den, prev[1][0], prev[1][1], grp - 1)
                        prev = ((at0, rels0), (at1, rels1))
                    av_group(b, 0, av0, den, prev[0][0], prev[0][1], ng - 1)
                    av_group(b, 1, av0, den, prev[1][0], prev[1][1], ng - 1)
                    for f, _cost, _due in fillers:
                        f()
                    if final:
                        # split norm so the first transpose pair starts as
                        # soon as the first half of ccp is normalized
                        rt = rt_pool.tile([128, 4, 2, 1], F32, tag="rt",
                                          name="rt")
                        nc.vector.reciprocal(rt, den[:, :, :, None])
                        ccp0 = ccp_pool.tile([128, 4, 2, 64], BF16,
                                             tag="ccp", name="ccp")
                        for hf in range(2):
                            nc.vector.tensor_tensor(
                                out=ccp0[:, 2 * hf:2 * hf + 2, :, :],
                                in0=av0[:, 2 * hf:2 * hf + 2, :, :],
                                in1=rt[:, 2 * hf:2 * hf + 2, :, :].to_broadcast(
                                    [128, 2, 2, 64]),
                                op=_AluOp.mult,
                            )
                    else:
                        ccp0 = norm_c(av0, den)

                    holder = {}

                    def trans_unit():
                        # 4 full-width PE transposes (qc-major ccp makes each
                        # a [128q,128f] -> [128f,128q] square), then one DVE
                        # copy psum->sbuf (split per half for the final block
                        # so oproj starts off the first half)
                        cc = ccs_pool.tile([128, 512], BF16, tag="cc",
                                           name="cc")
                        for hf in range(2):
                            for qc in (2 * hf, 2 * hf + 1):
                                nc.tensor.transpose(
                                    ct[:, qc * 128:(qc + 1) * 128],
                                    ccp0[:, qc, :, :].rearrange(
                                        "p a b -> p (a b)"),
                                    ident_sb,
                                )
                            if final:
                                nc.vector.tensor_copy(
                                    cc[:, hf * 256:hf * 256 + 256],
                                    ct[:, hf * 256:hf * 256 + 256])
                        if not final:
                            nc.vector.tensor_copy(cc, ct)
                        holder["cc"] = cc

                    def oproj2(n2, half):
                        cc = holder["cc"]
                        for ot in (2 * half, 2 * half + 1):
                            po = mmp.tile([128, 512], F32, tag="po", name="po")
                            nc.tensor.matmul(
                                po,
                                lhsT=cc[:, ot * 128:(ot + 1) * 128],
                                rhs=wo_sb[:, n2 * 512:(n2 + 1) * 512],
                                start=True, stop=True,
                            )
                            ob = ob_pool.tile([128, 512], BF16, tag="ob", name="ob")
                            if act and ot % 2 == 1:
                                nc.scalar.copy(ob, po)
                            else:
                                nc.vector.tensor_copy(ob, po)
                            q = sqs[s_i[0] % len(sqs)]
                            s_i[0] += 1
                            q.dma_start(
                                out=out[tok0 + ot * 128: tok0 + (ot + 1) * 128,
                                        n2 * 512:(n2 + 1) * 512],
                                in_=ob)

                    return [(trans_unit, 500.0, None)] + [
                        (lambda n2=n2, h2=h2: oproj2(n2, h2), 430.0, None)
                        for n2 in range(2) for h2 in range(2)]

                # proj units (~0.85us each), keyed for the drip schedule
                U = {}
                for g2 in range(16):
                    U[f"v{g2}"] = lambda g2=g2: proj_v2(g2)
                for ti in range(4, 8):
                    for hf, hn in ((0, "a"), (1, "b")):
                        U[f"q{ti}{hn}"] = lambda ti=ti, hf=hf: proj_qk_half("q", ti, hf)
                        U[f"k{ti}{hn}"] = lambda ti=ti, hf=hf: proj_qk_half("k", ti, hf)

                # drip allocation: units assigned to a block are guaranteed
                # emitted within it (deadline); the in-block budget logic
                # paces pops so PE work covers each group's exp time.
                # Batch-1 runs ascending so its K/V projections spread across
                # the batch-1 blocks; the final (1,2) block's P2 drains with
                # the scalar engine idle.
                takes = {
                    (0, 3): [(f"v{g}", g + 2) for g in range(7)] + [("v7", 8)],
                    (0, 2): [("q7a", None), ("q7b", None),
                             ("k4a", None), ("k4b", None)],
                    (0, 1): [("q6a", None), ("q6b", None),
                             ("k5a", None), ("k5b", None)],
                    (1, 3): [("v8", 1), ("v9", 2), ("v10", 3),
                             ("k6a", 4), ("k6b", 4), ("v11", 4),
                             ("v12", 5), ("k7a", 6), ("k7b", 6),
                             ("v13", 6), ("v14", 7), ("v15", 8)],
                    (1, 2): [("q5a", None), ("q5b", None)],
                    (0, 0): [],
                    (1, 1): [("q4a", None), ("q4b", None)],
                    (1, 0): [],
                }
                seq = [(0, 3), (0, 2), (0, 1), (1, 3), (1, 2),
                       (0, 0), (1, 1), (1, 0)]
                # the last blocks' stores spread across all three DMA
                # queues: at the drain the sync queue otherwise serializes
                # one store per ~700ns while scalar/gpsimd queues sit idle
                store_qs = {
                    (1, 2): [nc.sync, nc.gpsimd],
                    (1, 1): [nc.sync, nc.gpsimd],
                    (1, 0): [nc.scalar, nc.sync],
                }
                pending = []
                for b, qt in seq:
                    drip = pending + [(U[k], 853.0, due)
                                      for k, due in takes[(b, qt)]]
                    pending = attention(
                        b, qt, drip,
                        act=(b, qt) in [(1, 2), (1, 1), (1, 0)],
                        pop_start=1 if (b, qt) in [(0, 3), (0, 2)] else 0,
                        store_q=store_qs.get((b, qt)),
                        final=(b, qt) == (1, 0))
                for f, _cost, _due in pending:
                    f()
    return nc


_NC_CACHE = None


def _get_nc():
    global _NC_CACHE
    if _NC_CACHE is None:
        _NC_CACHE = build_nc()
        if not _NC_CACHE.is_finalized():
            _NC_CACHE.finalize()
    return _NC_CACHE


def _make_cmask():
    # at layout is [k, q]: valid (unmasked) iff k_rel <= q_rel
    p = np.arange(128)[:, None]
    f = np.arange(128)[None, :]
    return (p <= f).astype(NP_BF16)


def _shard_inputs(x, Wq, bq, Wk, bk, Wv, bv, Wo, bo):
    x = np.asarray(x, np.float32)
    Wq, Wk, Wv, Wo = (np.asarray(a, np.float32) for a in (Wq, Wk, Wv, Wo))
    bq, bk, bv = (np.asarray(a, np.float32) for a in (bq, bk, bv))

    xT = np.ascontiguousarray(x.reshape(T, D_MODEL).T).astype(NP_BF16)
    cmask = _make_cmask()
    ident = np.eye(128, dtype=NP_BF16)

    def wflat(W):
        # [1024, 128] -> [128, 8*128]: element (p, c*128+f) = W[c*128+p, f]
        return np.ascontiguousarray(
            W.reshape(KC, 128, FPC).transpose(1, 0, 2).reshape(128, KC * FPC)
        ).astype(NP_BF16)

    in_maps = []
    for c in range(N_CORES):
        fs = slice(c * FPC, (c + 1) * FPC)
        in_maps.append({
            "xT": xT,
            "wq": wflat(Wq[:, fs] / 8.0),
            "wk": wflat(Wk[:, fs]),
            "wv": wflat(Wv[:, fs]),
            "wo": np.ascontiguousarray(Wo[fs, :]).astype(NP_BF16),
            "bq": np.ascontiguousarray((bq[fs] / 8.0)[:, None]),
            "bk": np.ascontiguousarray(bk[fs][:, None]),
            "bv": np.ascontiguousarray(bv[fs]).reshape(1, FPC),
            "cmask": cmask,
            "ident": ident,
        })
    return in_maps


def _gather(results, bo):
    total = np.zeros((T, D_MODEL), np.float32)
    for c in range(N_CORES):
        total += np.asarray(results[c]["out"], np.float32)
    total += np.asarray(bo, np.float32)[None, :]
    return total.reshape(B, S, D_MODEL)


def kernel(x, Wq, bq, Wk, bk, Wv, bv, Wo, bo):
    in_maps = _shard_inputs(x, Wq, bq, Wk, bk, Wv, bv, Wo, bo)
    nc = _get_nc()
    res = run_bass_kernel_spmd(nc, in_maps, list(range(N_CORES)))
    return _gather(res.results, bo)


if __name__ == "__main__":
    rng = np.random.default_rng(0)
    x = rng.standard_normal((B, S, D_MODEL)).astype(np.float32)
    sc = 1 / np.sqrt(D_MODEL)
    args = dict(
        x=x,
        Wq=rng.standard_normal((D_MODEL, D_MODEL)).astype(np.float32) * sc,
        bq=np.zeros(D_MODEL, np.float32),
        Wk=rng.standard_normal((D_MODEL, D_MODEL)).astype(np.float32) * sc,
        bk=np.zeros(D_MODEL, np.float32),
        Wv=rng.standard_normal((D_MODEL, D_MODEL)).astype(np.float32) * sc,
        bv=np.zeros(D_MODEL, np.float32),
        Wo=rng.standard_normal((D_MODEL, D_MODEL)).astype(np.float32) * sc,
        bo=np.zeros(D_MODEL, np.float32),
    )
    out = kernel(**args)
    print("kernel output", out.shape, out.dtype, np.abs(out).max())
